# revision 1
# baseline (speedup 1.0000x reference)
"""Trainium2 8-core SPMD kernel for a 3-layer GIN network (GINConv x3 ->
global_add_pool -> Linear -> ReLU -> Linear).

Strategy (graph/edge partition per the sharding hint):
  - 100000 nodes padded to 100352; core c owns 12544 consecutive dst nodes
    (98 blocks of 128); each core processes the edges incident to its dst
    nodes. MLP weights replicated.
  - Node features H stored bf16 [100352, 128] (64 features + pad to 256B
    rows, pad bytes never read).
  - Neighbor gather: dma_gather (custom SWDGE gather ucode) with int16
    in-bank row indices over 4 row-banks of H, chunked ~512 rows/call.
  - Scatter-add per 128-dst block: PSUM[64,128] += msg.T @ M2 on the
    tensor engine, where M2[e,s] = (dstslot[e] == s) is built on the
    vector engine by comparing a per-edge dst-slot table against an iota
    constant. Self loops (z = h + agg) are synthetic edges with slot = p.
  - GIN MLPs run in fp32 on the PE (weights fp32; messages bf16).
  - After conv1/conv2: AllGather of the 12544-row shard -> full H.
  - conv3 feeds global_add_pool: PSUM[64,512] += h3.T @ B with B the
    batch one-hot (fp16 iota compare); AllReduce; the final linear head
    runs fp32 on-core; output [512, 1] fp32 taken from core 0.
"""

import numpy as np
import ml_dtypes

import concourse.bass as bass
import concourse.tile as tile
from concourse import bacc, mybir

BF16 = mybir.dt.bfloat16
F16 = mybir.dt.float16
F32 = mybir.dt.float32
I16 = mybir.dt.int16
AF = mybir.ActivationFunctionType
ALU = mybir.AluOpType

D = 64
ROW = 128
CORES = 8
NBANKS = 4
NP_PAD = 100352
N_GRAPHS = 512
N_NODES = 100000
SUPER = 4
GATHER_CHUNK = 512


class _Plan:
    pass


def _make_plan(src, dst, batch, n_real, NP, S=SUPER, cores=CORES):
    PER = NP // cores
    NBLK = PER // 128
    BANK = NP // NBANKS
    p = _Plan()
    p.NP, p.PER, p.NBLK, p.BANK, p.S, p.cores = NP, PER, NBLK, BANK, S, cores

    blk = (dst // 128).astype(np.int64)
    slot = (dst % 128).astype(np.int64)
    bank = (src // BANK).astype(np.int64)
    row = (src % BANK).astype(np.int64)
    core = (dst // PER).astype(np.int64)
    blk_local = blk - core * NBLK

    counts = np.zeros((cores, NBLK, NBANKS), np.int64)
    np.add.at(counts, (core, blk_local, bank), 1)
    self_bank = np.zeros((cores, NBLK), np.int64)
    for c in range(cores):
        for b in range(NBLK):
            sb = (c * PER + b * 128) // BANK
            self_bank[c, b] = sb
            counts[c, b, sb] += 128

    order = np.lexsort((bank, blk))
    srt_blk = blk[order]; srt_bank = bank[order]
    srt_row = row[order]; srt_slot = slot[order]
    key = srt_blk * NBANKS + srt_bank
    uniq, starts = np.unique(key, return_index=True)
    starts = np.append(starts, len(key))
    seg_of = {int(u): (int(s), int(e)) for u, s, e in zip(uniq, starts[:-1], starts[1:])}

    sb_list = []
    b0 = 0
    while b0 < NBLK:
        sb_list.append((b0, min(b0 + S, NBLK)))
        b0 += S
    p.superblocks = sb_list

    gather_cols = []
    sb_tiles = []
    sb_tile_base = []
    col_base = 0
    tile_base = 0
    seg_info = []
    for (b0, b1) in sb_list:
        glist = []
        toff = 0
        sinfo = []
        for k in range(NBANKS):
            tot = counts[:, b0:b1, k].sum(axis=1)
            ni = int(((tot.max() + 127) // 128) * 128)
            if ni == 0:
                continue
            cum = np.zeros((cores, b1 - b0 + 1), np.int64)
            cum[:, 1:] = np.cumsum(counts[:, b0:b1, k], axis=1)
            glist.append((k, ni, col_base, toff))
            sinfo.append((k, ni, toff, cum))
            col_base += ni // 16
            toff += ni // 128
        gather_cols.append(glist)
        seg_info.append(sinfo)
        sb_tiles.append(toff)
        sb_tile_base.append(tile_base)
        tile_base += toff
    p.gather_cols = gather_cols
    p.sb_tiles = sb_tiles
    p.sb_tile_base = sb_tile_base
    p.TS_max = max(sb_tiles)
    p.TT = tile_base
    p.NI_total = p.TT * 128
    p.COLS = col_base

    pair_base = 0
    p.block_pairs = []
    for ib, (b0, b1) in enumerate(sb_list):
        sinfo = seg_info[ib]
        for b in range(b0, b1):
            pl = []
            for (k, ni, toff, cum) in sinfo:
                lo = int(cum[:, b - b0].min())
                hi = int(cum[:, b - b0 + 1].max())
                if hi <= lo:
                    continue
                for t in range(lo // 128, (hi + 127) // 128):
                    pl.append((toff + t, k))
            p.block_pairs.append((b, pair_base, pl))
            pair_base += len(pl)
    p.NPAIRS = pair_base
    p.npairs_max = max(len(pl) for (_, _, pl) in p.block_pairs)

    idx_rows = np.zeros((cores, p.TT * 128), np.int16)
    slot_stream = np.full((cores, p.TT * 128), -1.0, np.float32)
    blkid_stream = np.full((cores, p.TT * 128), -1, np.int64)
    for c in range(cores):
        for ib, (b0, b1) in enumerate(sb_list):
            base_rows = sb_tile_base[ib] * 128
            for (k, ni, toff, cum) in seg_info[ib]:
                seg0 = base_rows + toff * 128
                for b in range(b0, b1):
                    pos = seg0 + int(cum[c, b - b0])
                    gblk = c * NBLK + b
                    if self_bank[c, b] == k:
                        nodes = np.arange(gblk * 128, gblk * 128 + 128)
                        idx_rows[c, pos:pos + 128] = (nodes % BANK).astype(np.int16)
                        slot_stream[c, pos:pos + 128] = np.arange(128)
                        blkid_stream[c, pos:pos + 128] = b
                        pos += 128
                    sk = seg_of.get(gblk * NBANKS + k)
                    if sk is not None:
                        s0, s1 = sk
                        idx_rows[c, pos:pos + (s1 - s0)] = srt_row[s0:s1].astype(np.int16)
                        slot_stream[c, pos:pos + (s1 - s0)] = srt_slot[s0:s1]
                        blkid_stream[c, pos:pos + (s1 - s0)] = b
                        pos += s1 - s0

    pair_slots = np.full((cores, 128, p.NPAIRS), -1.0, np.float32)
    for c in range(cores):
        for (b, pb, pl) in p.block_pairs:
            ib = b // S
            base_rows = sb_tile_base[ib] * 128
            for j, (t_in_sb, k) in enumerate(pl):
                r0 = base_rows + t_in_sb * 128
                rows_slots = slot_stream[c, r0:r0 + 128]
                rows_blk = blkid_stream[c, r0:r0 + 128]
                pair_slots[c, :, pb + j] = np.where(rows_blk == b, rows_slots, -1.0)

    idx_wrapped = np.zeros((cores, 128, p.COLS), np.int16)
    for ib, glist in enumerate(gather_cols):
        for (k, ni, col_off, toff) in glist:
            i0 = (sb_tile_base[ib] + toff) * 128
            for c in range(cores):
                w = idx_rows[c, i0:i0 + ni].reshape(ni // 16, 16).T
                idx_wrapped[c, :, col_off:col_off + ni // 16] = np.tile(w, (8, 1))

    p.idx_wrapped = idx_wrapped
    p.dstslot = pair_slots.astype(ml_dtypes.bfloat16)

    batch_pad = np.full(NP, -1.0, np.float32)
    batch_pad[:n_real] = batch.astype(np.float32)
    batchslot = np.empty((cores, 128, NBLK), np.float16)
    for c in range(cores):
        bs = batch_pad[c * PER:(c + 1) * PER].reshape(NBLK, 128).T
        batchslot[c] = bs.astype(np.float16)
    p.batchslot = batchslot
    return p


def _prep_inputs(p, x, weights):
    NP = p.NP
    h0 = np.zeros((NP, ROW), ml_dtypes.bfloat16)
    h0[:x.shape[0], :D] = x.astype(ml_dtypes.bfloat16)

    iota128 = np.broadcast_to(np.arange(128, dtype=np.float32), (128, 128)).astype(ml_dtypes.bfloat16)
    iotaG = np.broadcast_to(np.arange(N_GRAPHS, dtype=np.float32), (128, N_GRAPHS)).astype(np.float16)

    shared = {
        "h0": h0,
        "iota128": np.ascontiguousarray(iota128),
        "iotag": np.ascontiguousarray(iotaG),
        "ones_row": np.ones((1, 128), np.float32),
    }
    for i in (1, 2, 3):
        shared[f"c{i}w1"] = weights[f"conv{i}_w1"].astype(np.float32)
        shared[f"c{i}b1"] = weights[f"conv{i}_b1"].astype(np.float32).reshape(D, 1)
        shared[f"c{i}w2"] = weights[f"conv{i}_w2"].astype(np.float32)
        shared[f"c{i}b2"] = weights[f"conv{i}_b2"].astype(np.float32).reshape(1, D)
    shared["l1w"] = weights["lin1_w"].astype(np.float32)
    shared["l1b"] = weights["lin1_b"].astype(np.float32).reshape(D, 1)
    shared["l2w"] = weights["lin2_w"].astype(np.float32)
    shared["l2b"] = weights["lin2_b"].astype(np.float32).reshape(1, 1)

    in_maps = []
    for c in range(p.cores):
        m = dict(shared)
        m["idx"] = np.ascontiguousarray(p.idx_wrapped[c])
        m["dslot"] = np.ascontiguousarray(p.dstslot[c])
        m["bslot"] = np.ascontiguousarray(p.batchslot[c])
        in_maps.append(m)
    return in_maps


def _build_nc(p, gather_chunk=GATHER_CHUNK):
    nc = bacc.Bacc("TRN2", target_bir_lowering=False, debug=False,
                   num_devices=p.cores)
    NP, PER, NBLK, BANK = p.NP, p.PER, p.NBLK, p.BANK
    G = N_GRAPHS

    h0 = nc.dram_tensor("h0", [NP, ROW], BF16, kind="ExternalInput")
    idx_d = nc.dram_tensor("idx", [128, p.COLS], I16, kind="ExternalInput")
    dslot_d = nc.dram_tensor("dslot", [128, p.NPAIRS], BF16, kind="ExternalInput")
    bslot_d = nc.dram_tensor("bslot", [128, NBLK], F16, kind="ExternalInput")
    iota128_d = nc.dram_tensor("iota128", [128, 128], BF16, kind="ExternalInput")
    iotag_d = nc.dram_tensor("iotag", [128, G], F16, kind="ExternalInput")
    ones_d = nc.dram_tensor("ones_row", [1, 128], F32, kind="ExternalInput")
    wd = {}
    for i in (1, 2, 3):
        wd[f"c{i}w1"] = nc.dram_tensor(f"c{i}w1", [D, D], F32, kind="ExternalInput")
        wd[f"c{i}b1"] = nc.dram_tensor(f"c{i}b1", [D, 1], F32, kind="ExternalInput")
        wd[f"c{i}w2"] = nc.dram_tensor(f"c{i}w2", [D, D], F32, kind="ExternalInput")
        wd[f"c{i}b2"] = nc.dram_tensor(f"c{i}b2", [1, D], F32, kind="ExternalInput")
    l1w_d = nc.dram_tensor("l1w", [D, D], F32, kind="ExternalInput")
    l1b_d = nc.dram_tensor("l1b", [D, 1], F32, kind="ExternalInput")
    l2w_d = nc.dram_tensor("l2w", [D, 1], F32, kind="ExternalInput")
    l2b_d = nc.dram_tensor("l2b", [1, 1], F32, kind="ExternalInput")
    out_d = nc.dram_tensor("out", [1, G], F32, kind="ExternalOutput")

    rg = [list(range(p.cores))]

    with tile.TileContext(nc) as tc:
        with (
            tc.tile_pool(name="const", bufs=1) as cp,
            tc.tile_pool(name="msg", bufs=2) as msgp,
            tc.tile_pool(name="m2", bufs=2) as m2p,
            tc.tile_pool(name="work", bufs=3) as wp,
            tc.tile_pool(name="hout", bufs=3) as hop,
            tc.tile_pool(name="psA", bufs=2, space="PSUM") as psA,
            tc.tile_pool(name="psB", bufs=2, space="PSUM") as psB,
            tc.tile_pool(name="psC", bufs=2, space="PSUM") as psC,
            tc.tile_pool(name="psPool", bufs=1, space="PSUM") as psP,
            tc.tile_pool(name="dram", bufs=1, space="DRAM") as dp,
        ):
            idx_sb = cp.tile([128, p.COLS], I16)
            nc.sync.dma_start(idx_sb[:], idx_d[:])
            dslot_sb = cp.tile([128, p.NPAIRS], BF16)
            nc.sync.dma_start(dslot_sb[:], dslot_d[:])
            bslot_sb = cp.tile([128, NBLK], F16)
            nc.sync.dma_start(bslot_sb[:], bslot_d[:])
            iota128 = cp.tile([128, 128], BF16)
            nc.sync.dma_start(iota128[:], iota128_d[:])
            iotag = cp.tile([128, G], F16)
            nc.sync.dma_start(iotag[:], iotag_d[:])
            ones_sb = cp.tile([1, 128], F32)
            nc.sync.dma_start(ones_sb[:], ones_d[:])
            ws = {}
            for i in (1, 2, 3):
                for nm, shape in ((f"c{i}w1", [D, D]), (f"c{i}b1", [D, 1]),
                                  (f"c{i}w2", [D, D]), (f"c{i}b2", [1, D])):
                    ws[nm] = cp.tile(shape, F32, name=nm + "s")
                    nc.sync.dma_start(ws[nm][:], wd[nm][:])
            l1w = cp.tile([D, D], F32)
            nc.sync.dma_start(l1w[:], l1w_d[:])
            l1b = cp.tile([D, 1], F32)
            nc.sync.dma_start(l1b[:], l1b_d[:])
            l2w = cp.tile([D, 1], F32)
            nc.sync.dma_start(l2w[:], l2w_d[:])
            l2b = cp.tile([1, 1], F32)
            nc.sync.dma_start(l2b[:], l2b_d[:])

            h1_loc = dp.tile([PER, ROW], BF16)
            h2_loc = dp.tile([PER, ROW], BF16)
            h1_full = dp.tile([NP, ROW], BF16)
            h2_full = dp.tile([NP, ROW], BF16)
            pool_in = dp.tile([D, G], F32)
            pool_out = dp.tile([D, G], F32)

            pool_ps = psP.tile([D, G], F32, space="PSUM")

            def conv_layer(li, h_src, h_loc):
                w1, b1 = ws[f"c{li}w1"], ws[f"c{li}b1"]
                w2, b2 = ws[f"c{li}w2"], ws[f"c{li}b2"]
                for ib, (b0, b1blk) in enumerate(p.superblocks):
                    TS = p.sb_tiles[ib]
                    msg = msgp.tile([128, p.TS_max * ROW], BF16, tag="msg")
                    msgv = msg[:].rearrange("p (a b) -> p a b", b=ROW)
                    for (k, ni, col_off, toff) in p.gather_cols[ib]:
                        off = 0
                        while off < ni:
                            cni = min(gather_chunk, ni - off)
                            nc.gpsimd.dma_gather(
                                out_ap=msgv[:, toff + off // 128:toff + (off + cni) // 128, :],
                                in_ap=h_src[k * BANK:(k + 1) * BANK, :],
                                idxs_ap=idx_sb[:, col_off + off // 16:col_off + (off + cni) // 16],
                                num_idxs=cni,
                                num_idxs_reg=cni,
                                elem_size=ROW,
                                single_packet=True,
                            )
                            off += cni
                    for b in range(b0, b1blk):
                        _, pb, pl = p.block_pairs[b]
                        nbp = len(pl)
                        m2 = m2p.tile([128, p.npairs_max * 128], BF16, tag="m2")
                        m2v = m2[:].rearrange("p (a b) -> p a b", b=128)
                        nc.vector.tensor_tensor(
                            out=m2v[:, 0:nbp, :],
                            in0=dslot_sb[:, pb:pb + nbp].unsqueeze(2).broadcast_to((128, nbp, 128)),
                            in1=iota128[:].unsqueeze(1).broadcast_to((128, nbp, 128)),
                            op=ALU.is_equal,
                        )
                        agg = psA.tile([D, 128], F32, space="PSUM", tag="agg")
                        for j, (t_in_sb, _k) in enumerate(pl):
                            nc.tensor.matmul(
                                agg[:], lhsT=msgv[:, t_in_sb, 0:D],
                                rhs=m2v[:, j, :],
                                start=(j == 0), stop=(j == nbp - 1),
                            )
                        z = wp.tile([D, 128], F32, tag="z")
                        nc.vector.tensor_copy(z[:], agg[:])
                        ps1 = psB.tile([D, 128], F32, space="PSUM", tag="mlp1")
                        nc.tensor.matmul(ps1[:], lhsT=w1[:], rhs=z[:],
                                         start=True, stop=True)
                        a1 = wp.tile([D, 128], F32, tag="a1")
                        nc.scalar.activation(a1[:], ps1[:], AF.Relu, bias=b1[:])
                        ps2 = psC.tile([128, D], F32, space="PSUM", tag="mlp2")
                        nc.tensor.matmul(ps2[:], lhsT=a1[:], rhs=w2[:],
                                         start=True, stop=False)
                        nc.tensor.matmul(ps2[:], lhsT=ones_sb[:], rhs=b2[:],
                                         start=False, stop=True)
                        h3 = hop.tile([128, D], BF16, tag="h3")
                        nc.scalar.activation(h3[:], ps2[:], AF.Relu)
                        if h_loc is not None:
                            nc.sync.dma_start(h_loc[b * 128:(b + 1) * 128, 0:D], h3[:])
                        else:
                            B = wp.tile([128, G], BF16, tag="bsel")
                            nc.vector.tensor_tensor(
                                out=B[:],
                                in0=bslot_sb[:, b:b + 1].to_broadcast((128, G)),
                                in1=iotag[:],
                                op=ALU.is_equal,
                            )
                            nc.tensor.matmul(pool_ps[:], lhsT=h3[:], rhs=B[:],
                                             start=(b == 0), stop=(b == NBLK - 1),
                                             skip_group_check=True)

            conv_layer(1, h0[:], h1_loc[:])
            nc.gpsimd.collective_compute(
                "AllGather", ALU.bypass, replica_groups=rg,
                ins=[h1_loc.opt()], outs=[h1_full.opt()])
            conv_layer(2, h1_full[:], h2_loc[:])
            nc.gpsimd.collective_compute(
                "AllGather", ALU.bypass, replica_groups=rg,
                ins=[h2_loc.opt()], outs=[h2_full.opt()])
            conv_layer(3, h2_full[:], None)

            pool_sb = wp.tile([D, G], F32, tag="poolsb")
            nc.vector.tensor_copy(pool_sb[:], pool_ps[:])
            nc.sync.dma_start(pool_in[:], pool_sb[:])
            nc.gpsimd.collective_compute(
                "AllReduce", ALU.add, replica_groups=rg,
                ins=[pool_in.opt()], outs=[pool_out.opt()])
            pool_red = wp.tile([D, G], F32, tag="poolred")
            nc.sync.dma_start(pool_red[:], pool_out[:])
            ph1 = psB.tile([D, G], F32, space="PSUM", tag="mlp1")
            nc.tensor.matmul(ph1[:], lhsT=l1w[:], rhs=pool_red[:],
                             start=True, stop=True)
            s1 = wp.tile([D, G], F32, tag="s1")
            nc.scalar.activation(s1[:], ph1[:], AF.Relu, bias=l1b[:])
            ph2 = psC.tile([1, G], F32, space="PSUM", tag="mlp2")
            nc.tensor.matmul(ph2[:], lhsT=l2w[:], rhs=s1[:],
                             start=True, stop=True)
            og = wp.tile([1, G], F32, tag="og")
            nc.scalar.activation(og[:], ph2[:], AF.Identity, bias=l2b[:])
            nc.sync.dma_start(out_d[:], og[:])

    nc.compile()
    return nc


def kernel(**inputs):
    x = np.asarray(inputs["x"], np.float32)
    ei = np.asarray(inputs["edge_index"], np.int64)
    src, dst = ei[0], ei[1]
    batch = np.asarray(inputs["batch"], np.int64)
    weights = {k: np.asarray(v, np.float32) for k, v in inputs.items()
               if k not in ("x", "edge_index", "batch")}

    p = _make_plan(src, dst, batch, x.shape[0], NP_PAD)
    in_maps = _prep_inputs(p, x, weights)
    nc = _build_nc(p)

    from concourse import bass_utils
    res = bass_utils.run_bass_kernel_spmd(nc, in_maps, core_ids=list(range(CORES)))
    out = res.results[0]["out"]
    return out.reshape(-1)[:N_GRAPHS].reshape(N_GRAPHS, 1).astype(np.float32)



# revision 9
# speedup vs baseline: 1.0176x; 1.0176x over previous
"""Trainium2 8-core SPMD kernel for a 3-layer GIN network (GINConv x3 ->
global_add_pool -> Linear -> ReLU -> Linear).

Strategy (graph/edge partition per the sharding hint):
  - 100000 nodes padded to 100352; core c owns 12544 consecutive dst nodes
    (98 blocks of 128); each core processes the edges incident to its dst
    nodes. MLP weights replicated.
  - Node features H stored bf16 [100352, 128] (64 features + pad to 256B
    rows, pad bytes never read).
  - Neighbor gather: dma_gather (custom SWDGE gather ucode) with int16
    in-bank row indices over 4 row-banks of H, chunked ~512 rows/call.
  - Scatter-add per 128-dst block: PSUM[64,128] += msg.T @ M2 on the
    tensor engine, where M2[e,s] = (dstslot[e] == s) is built on the
    vector engine by comparing a per-edge dst-slot table against an iota
    constant. Self loops (z = h + agg) are synthetic edges with slot = p.
  - GIN MLPs run in fp32 on the PE (weights fp32; messages bf16).
  - After conv1/conv2: AllGather of the 12544-row shard -> full H.
  - conv3 feeds global_add_pool: PSUM[64,512] += h3.T @ B with B the
    batch one-hot (fp16 iota compare); AllReduce; the final linear head
    runs fp32 on-core; output [512, 1] fp32 taken from core 0.
"""

import numpy as np
import ml_dtypes

import concourse.bass as bass
import concourse.tile as tile
from concourse import bacc, mybir

BF16 = mybir.dt.bfloat16
F16 = mybir.dt.float16
F32 = mybir.dt.float32
I16 = mybir.dt.int16
AF = mybir.ActivationFunctionType
ALU = mybir.AluOpType

D = 64
ROW = 128
CORES = 8
NBANKS = 4
NP_PAD = 100352
N_GRAPHS = 512
N_NODES = 100000
SUPER = 4
GATHER_CHUNK = 512
NQUEUES = 1


class _Plan:
    pass


def _make_plan(src, dst, batch, n_real, NP, S=SUPER, cores=CORES):
    PER = NP // cores
    NBLK = PER // 128
    BANK = NP // NBANKS
    p = _Plan()
    p.NP, p.PER, p.NBLK, p.BANK, p.S, p.cores = NP, PER, NBLK, BANK, S, cores

    blk = (dst // 128).astype(np.int64)
    slot = (dst % 128).astype(np.int64)
    bank = (src // BANK).astype(np.int64)
    row = (src % BANK).astype(np.int64)
    core = (dst // PER).astype(np.int64)
    blk_local = blk - core * NBLK

    counts = np.zeros((cores, NBLK, NBANKS), np.int64)
    np.add.at(counts, (core, blk_local, bank), 1)
    self_bank = np.zeros((cores, NBLK), np.int64)
    for c in range(cores):
        for b in range(NBLK):
            sb = (c * PER + b * 128) // BANK
            self_bank[c, b] = sb
            counts[c, b, sb] += 128

    order = np.lexsort((bank, blk))
    srt_blk = blk[order]; srt_bank = bank[order]
    srt_row = row[order]; srt_slot = slot[order]
    key = srt_blk * NBANKS + srt_bank
    uniq, starts = np.unique(key, return_index=True)
    starts = np.append(starts, len(key))
    seg_of = {int(u): (int(s), int(e)) for u, s, e in zip(uniq, starts[:-1], starts[1:])}

    sb_list = []
    b0 = 0
    while b0 < NBLK:
        sb_list.append((b0, min(b0 + S, NBLK)))
        b0 += S
    p.superblocks = sb_list

    gather_cols = []
    sb_tiles = []
    sb_tile_base = []
    col_base = 0
    tile_base = 0
    seg_info = []
    for (b0, b1) in sb_list:
        glist = []
        toff = 0
        sinfo = []
        for k in range(NBANKS):
            tot = counts[:, b0:b1, k].sum(axis=1)
            ni = int(((tot.max() + 127) // 128) * 128)
            if ni == 0:
                continue
            cum = np.zeros((cores, b1 - b0 + 1), np.int64)
            cum[:, 1:] = np.cumsum(counts[:, b0:b1, k], axis=1)
            glist.append((k, ni, col_base, toff))
            sinfo.append((k, ni, toff, cum))
            col_base += ni // 16
            toff += ni // 128
        gather_cols.append(glist)
        seg_info.append(sinfo)
        sb_tiles.append(toff)
        sb_tile_base.append(tile_base)
        tile_base += toff
    p.gather_cols = gather_cols
    p.sb_tiles = sb_tiles
    p.sb_tile_base = sb_tile_base
    p.TS_max = max(sb_tiles)
    p.TT = tile_base
    p.NI_total = p.TT * 128
    p.COLS = col_base

    pair_base = 0
    p.block_pairs = []
    for ib, (b0, b1) in enumerate(sb_list):
        sinfo = seg_info[ib]
        for b in range(b0, b1):
            pl = []
            for (k, ni, toff, cum) in sinfo:
                lo = int(cum[:, b - b0].min())
                hi = int(cum[:, b - b0 + 1].max())
                if hi <= lo:
                    continue
                for t in range(lo // 128, (hi + 127) // 128):
                    pl.append((toff + t, k))
            p.block_pairs.append((b, pair_base, pl))
            pair_base += len(pl)
    p.NPAIRS = pair_base
    p.npairs_max = max(len(pl) for (_, _, pl) in p.block_pairs)

    idx_rows = np.zeros((cores, p.TT * 128), np.int16)
    slot_stream = np.full((cores, p.TT * 128), -1.0, np.float32)
    blkid_stream = np.full((cores, p.TT * 128), -1, np.int64)
    for c in range(cores):
        for ib, (b0, b1) in enumerate(sb_list):
            base_rows = sb_tile_base[ib] * 128
            for (k, ni, toff, cum) in seg_info[ib]:
                seg0 = base_rows + toff * 128
                for b in range(b0, b1):
                    pos = seg0 + int(cum[c, b - b0])
                    gblk = c * NBLK + b
                    if self_bank[c, b] == k:
                        nodes = np.arange(gblk * 128, gblk * 128 + 128)
                        idx_rows[c, pos:pos + 128] = (nodes % BANK).astype(np.int16)
                        slot_stream[c, pos:pos + 128] = np.arange(128)
                        blkid_stream[c, pos:pos + 128] = b
                        pos += 128
                    sk = seg_of.get(gblk * NBANKS + k)
                    if sk is not None:
                        s0, s1 = sk
                        idx_rows[c, pos:pos + (s1 - s0)] = srt_row[s0:s1].astype(np.int16)
                        slot_stream[c, pos:pos + (s1 - s0)] = srt_slot[s0:s1]
                        blkid_stream[c, pos:pos + (s1 - s0)] = b
                        pos += s1 - s0

    pair_slots = np.full((cores, 128, p.NPAIRS), -1.0, np.float32)
    for c in range(cores):
        for (b, pb, pl) in p.block_pairs:
            ib = b // S
            base_rows = sb_tile_base[ib] * 128
            for j, (t_in_sb, k) in enumerate(pl):
                r0 = base_rows + t_in_sb * 128
                rows_slots = slot_stream[c, r0:r0 + 128]
                rows_blk = blkid_stream[c, r0:r0 + 128]
                pair_slots[c, :, pb + j] = np.where(rows_blk == b, rows_slots, -1.0)

    idx_wrapped = np.zeros((cores, 128, p.COLS), np.int16)
    for ib, glist in enumerate(gather_cols):
        for (k, ni, col_off, toff) in glist:
            i0 = (sb_tile_base[ib] + toff) * 128
            for c in range(cores):
                w = idx_rows[c, i0:i0 + ni].reshape(ni // 16, 16).T
                idx_wrapped[c, :, col_off:col_off + ni // 16] = np.tile(w, (8, 1))

    p.idx_wrapped = idx_wrapped
    p.dstslot = pair_slots.astype(ml_dtypes.bfloat16)

    batch_pad = np.full(NP, -1.0, np.float32)
    batch_pad[:n_real] = batch.astype(np.float32)
    batchslot = np.empty((cores, 128, NBLK), np.float16)
    for c in range(cores):
        bs = batch_pad[c * PER:(c + 1) * PER].reshape(NBLK, 128).T
        batchslot[c] = bs.astype(np.float16)
    p.batchslot = batchslot
    return p


def _prep_inputs(p, x, weights):
    NP = p.NP
    h0 = np.zeros((NP, ROW), ml_dtypes.bfloat16)
    h0[:x.shape[0], :D] = x.astype(ml_dtypes.bfloat16)

    iota128 = np.broadcast_to(np.arange(128, dtype=np.float32), (128, 128)).astype(ml_dtypes.bfloat16)
    iotaG = np.broadcast_to(np.arange(N_GRAPHS, dtype=np.float32), (128, N_GRAPHS)).astype(np.float16)

    shared = {
        "h0": h0,
        "iota128": np.ascontiguousarray(iota128),
        "iotag": np.ascontiguousarray(iotaG),
        "ones_row": np.ones((1, 128), np.float32),
    }
    for i in (1, 2, 3):
        shared[f"c{i}w1"] = weights[f"conv{i}_w1"].astype(np.float32)
        shared[f"c{i}b1"] = weights[f"conv{i}_b1"].astype(np.float32).reshape(D, 1)
        shared[f"c{i}w2"] = weights[f"conv{i}_w2"].astype(np.float32)
        shared[f"c{i}b2"] = weights[f"conv{i}_b2"].astype(np.float32).reshape(1, D)
    shared["l1w"] = weights["lin1_w"].astype(np.float32)
    shared["l1b"] = weights["lin1_b"].astype(np.float32).reshape(D, 1)
    shared["l2w"] = weights["lin2_w"].astype(np.float32)
    shared["l2b"] = weights["lin2_b"].astype(np.float32).reshape(1, 1)

    in_maps = []
    for c in range(p.cores):
        m = dict(shared)
        m["idx"] = np.ascontiguousarray(p.idx_wrapped[c])
        m["dslot"] = np.ascontiguousarray(p.dstslot[c])
        m["bslot"] = np.ascontiguousarray(p.batchslot[c])
        in_maps.append(m)
    return in_maps


def _build_nc(p, gather_chunk=GATHER_CHUNK):
    nc = bacc.Bacc("TRN2", target_bir_lowering=False, debug=False,
                   num_devices=p.cores, num_swdge_queues=NQUEUES)
    NP, PER, NBLK, BANK = p.NP, p.PER, p.NBLK, p.BANK
    G = N_GRAPHS

    h0 = nc.dram_tensor("h0", [NP, ROW], BF16, kind="ExternalInput")
    idx_d = nc.dram_tensor("idx", [128, p.COLS], I16, kind="ExternalInput")
    dslot_d = nc.dram_tensor("dslot", [128, p.NPAIRS], BF16, kind="ExternalInput")
    bslot_d = nc.dram_tensor("bslot", [128, NBLK], F16, kind="ExternalInput")
    iota128_d = nc.dram_tensor("iota128", [128, 128], BF16, kind="ExternalInput")
    iotag_d = nc.dram_tensor("iotag", [128, G], F16, kind="ExternalInput")
    ones_d = nc.dram_tensor("ones_row", [1, 128], F32, kind="ExternalInput")
    wd = {}
    for i in (1, 2, 3):
        wd[f"c{i}w1"] = nc.dram_tensor(f"c{i}w1", [D, D], F32, kind="ExternalInput")
        wd[f"c{i}b1"] = nc.dram_tensor(f"c{i}b1", [D, 1], F32, kind="ExternalInput")
        wd[f"c{i}w2"] = nc.dram_tensor(f"c{i}w2", [D, D], F32, kind="ExternalInput")
        wd[f"c{i}b2"] = nc.dram_tensor(f"c{i}b2", [1, D], F32, kind="ExternalInput")
    l1w_d = nc.dram_tensor("l1w", [D, D], F32, kind="ExternalInput")
    l1b_d = nc.dram_tensor("l1b", [D, 1], F32, kind="ExternalInput")
    l2w_d = nc.dram_tensor("l2w", [D, 1], F32, kind="ExternalInput")
    l2b_d = nc.dram_tensor("l2b", [1, 1], F32, kind="ExternalInput")
    out_d = nc.dram_tensor("out", [1, G], F32, kind="ExternalOutput")

    rg = [list(range(p.cores))]

    with tile.TileContext(nc) as tc:
        with (
            tc.tile_pool(name="const", bufs=1) as cp,
            tc.tile_pool(name="msg", bufs=2) as msgp,
            tc.tile_pool(name="m2", bufs=2) as m2p,
            tc.tile_pool(name="work", bufs=3) as wp,
            tc.tile_pool(name="hout", bufs=3) as hop,
            tc.tile_pool(name="psA", bufs=2, space="PSUM") as psA,
            tc.tile_pool(name="psB", bufs=2, space="PSUM") as psB,
            tc.tile_pool(name="psC", bufs=2, space="PSUM") as psC,
            tc.tile_pool(name="psPool", bufs=1, space="PSUM") as psP,
            tc.tile_pool(name="dram", bufs=1, space="DRAM") as dp,
        ):
            idx_sb = cp.tile([128, p.COLS], I16)
            nc.sync.dma_start(idx_sb[:], idx_d[:])
            dslot_sb = cp.tile([128, p.NPAIRS], BF16)
            nc.sync.dma_start(dslot_sb[:], dslot_d[:])
            bslot_sb = cp.tile([128, NBLK], F16)
            nc.sync.dma_start(bslot_sb[:], bslot_d[:])
            iota128 = cp.tile([128, 128], BF16)
            nc.sync.dma_start(iota128[:], iota128_d[:])
            iotag = cp.tile([128, G], F16)
            nc.sync.dma_start(iotag[:], iotag_d[:])
            ones_sb = cp.tile([1, 128], F32)
            nc.sync.dma_start(ones_sb[:], ones_d[:])
            ws = {}
            for i in (1, 2, 3):
                for nm, shape in ((f"c{i}w1", [D, D]), (f"c{i}b1", [D, 1]),
                                  (f"c{i}w2", [D, D]), (f"c{i}b2", [1, D])):
                    ws[nm] = cp.tile(shape, F32, name=nm + "s")
                    nc.sync.dma_start(ws[nm][:], wd[nm][:])
            l1w = cp.tile([D, D], F32)
            nc.sync.dma_start(l1w[:], l1w_d[:])
            l1b = cp.tile([D, 1], F32)
            nc.sync.dma_start(l1b[:], l1b_d[:])
            l2w = cp.tile([D, 1], F32)
            nc.sync.dma_start(l2w[:], l2w_d[:])
            l2b = cp.tile([1, 1], F32)
            nc.sync.dma_start(l2b[:], l2b_d[:])

            h1_loc = dp.tile([PER, ROW], BF16)
            h2_loc = dp.tile([PER, ROW], BF16)
            h1_full = dp.tile([NP, ROW], BF16)
            h2_full = dp.tile([NP, ROW], BF16)
            pool_in = dp.tile([D, G], F32)
            pool_out = dp.tile([D, G], F32)

            pool_ps = psP.tile([D, G], F32, space="PSUM")

            def conv_layer(li, h_src, h_loc):
                w1, b1 = ws[f"c{li}w1"], ws[f"c{li}b1"]
                w2, b2 = ws[f"c{li}w2"], ws[f"c{li}b2"]
                for ib, (b0, b1blk) in enumerate(p.superblocks):
                    TS = p.sb_tiles[ib]
                    msg = msgp.tile([128, p.TS_max * ROW], BF16, tag="msg")
                    msgv = msg[:].rearrange("p (a b) -> p a b", b=ROW)
                    for (k, ni, col_off, toff) in p.gather_cols[ib]:
                        off = 0
                        while off < ni:
                            cni = min(gather_chunk, ni - off)
                            nc.gpsimd.dma_gather(
                                out_ap=msgv[:, toff + off // 128:toff + (off + cni) // 128, :],
                                in_ap=h_src[k * BANK:(k + 1) * BANK, :],
                                idxs_ap=idx_sb[:, col_off + off // 16:col_off + (off + cni) // 16],
                                num_idxs=cni,
                                num_idxs_reg=cni,
                                elem_size=ROW,
                                single_packet=True,
                                queue_num=k % NQUEUES,
                            )
                            off += cni
                    for b in range(b0, b1blk):
                        _, pb, pl = p.block_pairs[b]
                        nbp = len(pl)
                        m2 = m2p.tile([128, p.npairs_max * 128], BF16, tag="m2")
                        m2v = m2[:].rearrange("p (a b) -> p a b", b=128)
                        nc.vector.tensor_tensor(
                            out=m2v[:, 0:nbp, :],
                            in0=dslot_sb[:, pb:pb + nbp].unsqueeze(2).broadcast_to((128, nbp, 128)),
                            in1=iota128[:].unsqueeze(1).broadcast_to((128, nbp, 128)),
                            op=ALU.is_equal,
                        )
                        agg = psA.tile([D, 128], F32, space="PSUM", tag="agg")
                        for j, (t_in_sb, _k) in enumerate(pl):
                            nc.tensor.matmul(
                                agg[:], lhsT=msgv[:, t_in_sb, 0:D],
                                rhs=m2v[:, j, :],
                                start=(j == 0), stop=(j == nbp - 1),
                            )
                        z = wp.tile([D, 128], F32, tag="z")
                        nc.vector.tensor_copy(z[:], agg[:])
                        ps1 = psB.tile([D, 128], F32, space="PSUM", tag="mlp1")
                        nc.tensor.matmul(ps1[:], lhsT=w1[:], rhs=z[:],
                                         start=True, stop=True)
                        a1 = wp.tile([D, 128], F32, tag="a1")
                        nc.scalar.activation(a1[:], ps1[:], AF.Relu, bias=b1[:])
                        ps2 = psC.tile([128, D], F32, space="PSUM", tag="mlp2")
                        nc.tensor.matmul(ps2[:], lhsT=a1[:], rhs=w2[:],
                                         start=True, stop=False)
                        nc.tensor.matmul(ps2[:], lhsT=ones_sb[:], rhs=b2[:],
                                         start=False, stop=True)
                        h3 = hop.tile([128, D], BF16, tag="h3")
                        nc.scalar.activation(h3[:], ps2[:], AF.Relu)
                        if h_loc is not None:
                            nc.sync.dma_start(h_loc[b * 128:(b + 1) * 128, 0:D], h3[:])
                        else:
                            B = wp.tile([128, G], BF16, tag="bsel")
                            nc.vector.tensor_tensor(
                                out=B[:],
                                in0=bslot_sb[:, b:b + 1].to_broadcast((128, G)),
                                in1=iotag[:],
                                op=ALU.is_equal,
                            )
                            nc.tensor.matmul(pool_ps[:], lhsT=h3[:], rhs=B[:],
                                             start=(b == 0), stop=(b == NBLK - 1),
                                             skip_group_check=True)

            conv_layer(1, h0[:], h1_loc[:])
            nc.gpsimd.collective_compute(
                "AllGather", ALU.bypass, replica_groups=rg,
                ins=[h1_loc.opt()], outs=[h1_full.opt()])
            conv_layer(2, h1_full[:], h2_loc[:])
            nc.gpsimd.collective_compute(
                "AllGather", ALU.bypass, replica_groups=rg,
                ins=[h2_loc.opt()], outs=[h2_full.opt()])
            conv_layer(3, h2_full[:], None)

            pool_sb = wp.tile([D, G], F32, tag="poolsb")
            nc.vector.tensor_copy(pool_sb[:], pool_ps[:])
            nc.sync.dma_start(pool_in[:], pool_sb[:])
            nc.gpsimd.collective_compute(
                "AllReduce", ALU.add, replica_groups=rg,
                ins=[pool_in.opt()], outs=[pool_out.opt()])
            pool_red = wp.tile([D, G], F32, tag="poolred")
            nc.sync.dma_start(pool_red[:], pool_out[:])
            ph1 = psB.tile([D, G], F32, space="PSUM", tag="mlp1")
            nc.tensor.matmul(ph1[:], lhsT=l1w[:], rhs=pool_red[:],
                             start=True, stop=True)
            s1 = wp.tile([D, G], F32, tag="s1")
            nc.scalar.activation(s1[:], ph1[:], AF.Relu, bias=l1b[:])
            ph2 = psC.tile([1, G], F32, space="PSUM", tag="mlp2")
            nc.tensor.matmul(ph2[:], lhsT=l2w[:], rhs=s1[:],
                             start=True, stop=True)
            og = wp.tile([1, G], F32, tag="og")
            nc.scalar.activation(og[:], ph2[:], AF.Identity, bias=l2b[:])
            nc.sync.dma_start(out_d[:], og[:])

    nc.compile()
    return nc


def kernel(**inputs):
    x = np.asarray(inputs["x"], np.float32)
    ei = np.asarray(inputs["edge_index"], np.int64)
    src, dst = ei[0], ei[1]
    batch = np.asarray(inputs["batch"], np.int64)
    weights = {k: np.asarray(v, np.float32) for k, v in inputs.items()
               if k not in ("x", "edge_index", "batch")}

    p = _make_plan(src, dst, batch, x.shape[0], NP_PAD)
    in_maps = _prep_inputs(p, x, weights)
    nc = _build_nc(p)

    from concourse import bass_utils
    res = bass_utils.run_bass_kernel_spmd(nc, in_maps, core_ids=list(range(CORES)))
    out = res.results[0]["out"]
    return out.reshape(-1)[:N_GRAPHS].reshape(N_GRAPHS, 1).astype(np.float32)

